# revision 5
# baseline (speedup 1.0000x reference)
"""Binary-tree gated-expert MoE kernel for 8 Trainium2 NeuronCores.

Reference computation (B=4096, D=2048, 4 levels, 1/2/4/8 experts):
    h = x
    for level l: h = relu(h @ Wl[eid_l] + bl[eid_l])
where eid_l is the l-bit prefix of the 3-bit leaf id built from
path_mask[:, 0:3].

Strategy: expert-parallel over the 8 leaves with host-side dispatch.
Sorting samples by leaf id makes every level's expert groups contiguous
(level-l ids are prefixes of the leaf id), so core c processes leaf
group c and needs exactly 4 weight matrices: W0[0], W1[c>>2], W2[c>>1],
W3[c].  Groups are Binomial(B, 1/8) ~ 512+-21 rows; each is padded to a
common per-core batch Bc.  On device each core runs 4 chained
matmul+relu levels in feature-major layout (activations stay transposed
[D, Bc] in SBUF across all levels; output partition dim = output
features, so no transposes anywhere).

Everything runs in bfloat16 (weights, activations, output) with fp32
PSUM accumulation: same 1 cycle/column PE rate as float32r, but half
the HBM traffic (33.5 MB/core of weights vs 67) and hardware fast
weight load, so LDWEIGHTS hides completely under the matmul stream.
End-to-end quantization error ~5e-3 vs the 2e-2 gate.  Weights stream
HBM->SBUF per 512-column group, paced + multi-buffered.
"""

import math

import numpy as np
import ml_dtypes

from concourse import bacc, mybir, tile
from concourse.bass_utils import run_bass_kernel_spmd

D = 2048
KT = D // 128          # 16 contraction k-tiles
JT = D // 128          # 16 output-feature blocks
JG = 4                 # j-groups of 4 blocks (512 features) per W DMA
N_CORES = 8
N_LEVELS = 4
F32 = mybir.dt.float32
BF16 = mybir.dt.bfloat16
NPBF16 = ml_dtypes.bfloat16

_cache: dict = {}


def _build(Bc: int, chunk: int, nchunks: int, warm: int = 8):
    """Build + compile the per-core Bass program for batch Bc = chunk*nchunks."""
    key = (Bc, chunk, nchunks, warm)
    if key in _cache:
        return _cache[key]

    nc = bacc.Bacc("TRN2", target_bir_lowering=False, debug=False,
                   num_devices=N_CORES)

    # Weights arrive host-linearized as [JG, 128, KT*512]:
    # element (jg, p, kt, jc) = W[kt*128 + p, jg*512 + jc], so each DMA
    # reads long contiguous runs per partition.
    xT = nc.dram_tensor("xT", [D, Bc], BF16, kind="ExternalInput")
    Ws = [nc.dram_tensor(f"W{l}", [JG, 128, KT * 512], BF16,
                         kind="ExternalInput")
          for l in range(N_LEVELS)]
    bias = nc.dram_tensor("bias", [N_LEVELS, D], F32, kind="ExternalInput")
    out = nc.dram_tensor("out", [D, Bc], BF16, kind="ExternalOutput")

    xTv = xT.rearrange("(kt p) b -> p kt b", p=128)
    outv = out.rearrange("(jt p) b -> p jt b", p=128)
    bv = bias.rearrange("l (jt p) -> p l jt", p=128)
    NQ = 4                      # W DMA split: 4 quarters of 4 k-tiles
    KQ = KT // NQ               # k-tiles per quarter
    QW = KQ * 512               # W free-dim elements per quarter
    PACE_WIN = 3                # max in-flight paced DMAs on the SP ring

    with tile.TileContext(nc) as tc:
        with (
            tc.tile_pool(name="acts", bufs=1) as acts,
            tc.tile_pool(name="w", bufs=4) as wpool,
            tc.tile_pool(name="ps", bufs=8, space="PSUM") as ps,
            tc.tile_pool(name="misc", bufs=1) as misc,
        ):
            actA = acts.tile([128, KT, Bc], BF16, tag="A")
            actB = acts.tile([128, KT, Bc], BF16, tag="B")
            btile = misc.tile([128, N_LEVELS, JT], F32)
            nc.scalar.dma_start(btile[:], bv)

            # Warm the PE HAM clock gate during the DMA lead-in: a short
            # burst of throwaway matmuls on a zeroed tile so the first
            # real matmul doesn't start on an idle->throttled PE.
            warmt = misc.tile([128, 512], BF16)
            nc.gpsimd.memset(warmt[:], 0.0)
            wacc = ps.tile([128, 512], F32, tag="ps", name="wacc")
            for _ in range(warm):
                nc.tensor.matmul(wacc[:], warmt[:, :128], warmt[:],
                                 start=True, stop=True)

            # Bulk weight DMAs go on the SP HWDGE ring, chained so at
            # most PACE_WIN are in flight.  The HW SDMA engines
            # round-robin packets across every queued transfer, so an
            # unbounded backlog makes every transfer finish near the
            # end; a short chain keeps completion order = consumption
            # order with the stream still running at full HBM rate.
            # x streams in parallel on the Activation HWDGE ring so the
            # two never serialize behind each other during the lead-in.
            paced = []

            def paced_dma(dst_ap, src_ap):
                h = nc.sync.dma_start(dst_ap, src_ap)
                if len(paced) >= PACE_WIN:
                    tile.add_dep_helper(h.ins, paced[-PACE_WIN].ins,
                                        reason="dma pacing chain")
                paced.append(h)
                return h

            paced_x = []

            def paced_xdma(dst_ap, src_ap):
                h = nc.scalar.dma_start(dst_ap, src_ap)
                if len(paced_x) >= 2:
                    tile.add_dep_helper(h.ins, paced_x[-2].ins,
                                        reason="x pacing chain")
                paced_x.append(h)
                return h

            # x pieces in consumption-priority order: the first matmul
            # chain only needs k-quarter 0 of chunk 0, at k-tile grain.
            for kt in range(KQ):
                paced_xdma(actA[:, kt:kt + 1, 0:chunk],
                           xTv[:, kt:kt + 1, 0:chunk])
            if nchunks > 1:
                paced_xdma(actA[:, 0:KQ, chunk:Bc], xTv[:, 0:KQ, chunk:Bc])
            for q in range(1, NQ):
                ks = slice(q * KQ, (q + 1) * KQ)
                paced_xdma(actA[:, ks, :], xTv[:, ks, :])

            for l in range(N_LEVELS):
                src = actA if l % 2 == 0 else actB
                dst = actB if l % 2 == 0 else actA
                for jg in range(JG):
                    wt = wpool.tile([128, KT, 4 * 128], BF16, tag="w")
                    wflat = wt.rearrange("p kt j -> p (kt j)")
                    accs = [ps.tile([128, chunk], F32, tag="ps", name="acc")
                            for _ in range(nchunks * 4)]
                    last_jg = (l == N_LEVELS - 1 and jg == JG - 1)
                    for q in range(NQ):
                        if l == 0 and jg == 0 and q == 0:
                            # k-tile-grain pieces so the first matmul can
                            # start after ~128KB instead of ~256KB.
                            for kt in range(KQ):
                                paced_dma(
                                    wflat[:, kt * 512:(kt + 1) * 512],
                                    Ws[l][jg][:, kt * 512:(kt + 1) * 512])
                        else:
                            paced_dma(
                                wflat[:, q * QW:(q + 1) * QW],
                                Ws[l][jg][:, q * QW:(q + 1) * QW])
                        if last_jg:
                            continue  # emitted per-acc below
                        for c in range(nchunks):
                            cs = slice(c * chunk, (c + 1) * chunk)
                            for jj in range(4):
                                acc = accs[c * 4 + jj]
                                for kt in range(q * KQ, (q + 1) * KQ):
                                    nc.tensor.matmul(
                                        acc[:],
                                        wt[:, kt, jj * 128:(jj + 1) * 128],
                                        src[:, kt, cs],
                                        start=(kt == 0),
                                        stop=(kt == KT - 1),
                                    )
                    if not last_jg:
                        for c in range(nchunks):
                            cs = slice(c * chunk, (c + 1) * chunk)
                            for jj in range(4):
                                jt = jg * 4 + jj
                                acc = accs[c * 4 + jj]
                                nc.scalar.activation(
                                    dst[:, jt, cs], acc[:],
                                    mybir.ActivationFunctionType.Relu,
                                    bias=btile[:, l, jt:jt + 1],
                                )
                        if l == N_LEVELS - 1:
                            # final level, non-last jg: ship this jg's four
                            # feature blocks via SWDGE (GpSimd) so the
                            # store never head-of-line-blocks the paced W
                            # chain.
                            nc.gpsimd.dma_start(
                                outv[:, jg * 4:(jg + 1) * 4, :],
                                dst[:, jg * 4:(jg + 1) * 4, :])
                    else:
                        # Last jg of the last level: run each accumulation
                        # chain to completion so its activation + store
                        # overlap the remaining chains, shrinking the tail
                        # after the final matmul to ~1us.  The very last
                        # chain's activation + store are split in halves
                        # so the store of half 0 overlaps the activation
                        # of half 1; final stores ride the (now idle) SP
                        # HWDGE ring.
                        for c in range(nchunks):
                            cs = slice(c * chunk, (c + 1) * chunk)
                            for jj in range(4):
                                acc = accs[c * 4 + jj]
                                for kt in range(KT):
                                    nc.tensor.matmul(
                                        acc[:],
                                        wt[:, kt, jj * 128:(jj + 1) * 128],
                                        src[:, kt, cs],
                                        start=(kt == 0),
                                        stop=(kt == KT - 1),
                                    )
                                jt = jg * 4 + jj
                                last = (c == nchunks - 1 and jj == 3)
                                if not last:
                                    nc.scalar.activation(
                                        dst[:, jt, cs], acc[:],
                                        mybir.ActivationFunctionType.Relu,
                                        bias=btile[:, l, jt:jt + 1],
                                    )
                                    nc.gpsimd.dma_start(outv[:, jt, cs],
                                                        dst[:, jt, cs])
                                else:
                                    half = chunk // 2
                                    for h in range(2):
                                        hs = slice(h * half,
                                                   chunk if h else half)
                                        gcs = slice(c * chunk + hs.start,
                                                    c * chunk + hs.stop)
                                        nc.scalar.activation(
                                            dst[:, jt, gcs], acc[:, hs],
                                            mybir.ActivationFunctionType.Relu,
                                            bias=btile[:, l, jt:jt + 1],
                                        )
                                        nc.sync.dma_start(outv[:, jt, gcs],
                                                          dst[:, jt, gcs])

    nc.compile()
    _cache[key] = nc
    return nc


def _linearize_w(W: np.ndarray) -> np.ndarray:
    """[D, D] f32 -> bf16 [JG, 128, KT*512], (jg,p,kt,jc) = W[kt*128+p, jg*512+jc]."""
    return np.ascontiguousarray(
        W.astype(NPBF16).reshape(KT, 128, JG, 512).transpose(2, 1, 0, 3).reshape(
            JG, 128, KT * 512))


def _plan(path_mask: np.ndarray):
    pm = np.asarray(path_mask)
    e3 = (pm[:, 0] * 4 + pm[:, 1] * 2 + pm[:, 2]).astype(np.int64)
    counts = np.bincount(e3, minlength=N_CORES)
    maxg = int(max(counts.max(), 1))
    nchunks = max(1, math.ceil(maxg / 512))
    chunk = max(256, math.ceil(maxg / nchunks))
    chunk = min(512, (chunk + 7) // 8 * 8)
    Bc = chunk * nchunks
    return e3, maxg, Bc, chunk, nchunks


def kernel(x, path_mask, W0, b0, W1, b1, W2, b2, W3, b3, _trace=False):
    x = np.ascontiguousarray(np.asarray(x, dtype=np.float32))
    Wls = [np.asarray(W, dtype=np.float32) for W in (W0, W1, W2, W3)]
    bls = [np.asarray(b, dtype=np.float32) for b in (b0, b1, b2, b3)]
    B = x.shape[0]

    e3, maxg, Bc, chunk, nchunks = _plan(path_mask)
    if Bc > 672:
        # extreme routing skew: SBUF can't hold the activations in one
        # pass; fall back to multiple 512-row passes per core.
        Bc, chunk, nchunks = 512, 512, 1
    nseg = math.ceil(maxg / Bc)
    nc = _build(Bc, chunk, nchunks)

    core_rows = [np.nonzero(e3 == c)[0] for c in range(N_CORES)]
    wb_maps = []
    for c in range(N_CORES):
        eids = (0, c >> 2, c >> 1, c)
        wb_maps.append({
            **{f"W{l}": _linearize_w(Wls[l][eids[l]])
               for l in range(N_LEVELS)},
            "bias": np.ascontiguousarray(
                np.stack([bls[l][eids[l]] for l in range(N_LEVELS)])),
        })

    out_full = np.zeros((B, D), dtype=np.float32)
    last_res = None
    for s in range(nseg):
        in_maps = []
        for c in range(N_CORES):
            rows = core_rows[c][s * Bc:(s + 1) * Bc]
            xTc = np.zeros((D, Bc), dtype=NPBF16)
            xTc[:, :len(rows)] = x[rows].astype(NPBF16).T
            in_maps.append({"xT": xTc, **wb_maps[c]})
        res = run_bass_kernel_spmd(nc, in_maps, list(range(N_CORES)),
                                   trace=_trace)
        last_res = res
        for c in range(N_CORES):
            rows = core_rows[c][s * Bc:(s + 1) * Bc]
            out_full[rows] = res.results[c]["out"][:, :len(rows)].T.astype(
                np.float32)
    if _trace:
        return out_full, last_res
    return out_full


# revision 15
# speedup vs baseline: 1.0507x; 1.0507x over previous
"""Binary-tree gated-expert MoE kernel for 8 Trainium2 NeuronCores.

Reference computation (B=4096, D=2048, 4 levels, 1/2/4/8 experts):
    h = x
    for level l: h = relu(h @ Wl[eid_l] + bl[eid_l])
where eid_l is the l-bit prefix of the 3-bit leaf id built from
path_mask[:, 0:3].

Strategy: expert-parallel over the 8 leaves with host-side dispatch.
Sorting samples by leaf id makes every level's expert groups contiguous
(level-l ids are prefixes of the leaf id), so core c processes leaf
group c and needs exactly 4 weight matrices: W0[0], W1[c>>2], W2[c>>1],
W3[c].  Groups are Binomial(B, 1/8) ~ 512+-21 rows; each is padded to a
common per-core batch Bc.  On device each core runs 4 chained
matmul+relu levels in feature-major layout (activations stay transposed
[D, Bc] in SBUF across all levels; output partition dim = output
features, so no transposes anywhere).

Everything runs in bfloat16 (weights, activations, output) with fp32
PSUM accumulation: same 1 cycle/column PE rate as float32r, but half
the HBM traffic (33.5 MB/core of weights vs 67) and hardware fast
weight load, so LDWEIGHTS hides completely under the matmul stream.
End-to-end quantization error ~5e-3 vs the 2e-2 gate.  Weights stream
HBM->SBUF per 512-column group, paced + multi-buffered.
"""

import math

import numpy as np
import ml_dtypes

from concourse import bacc, mybir, tile
from concourse.bass_utils import run_bass_kernel_spmd

D = 2048
KT = D // 128          # 16 contraction k-tiles
JT = D // 128          # 16 output-feature blocks
JG = 4                 # j-groups of 4 blocks (512 features) per W DMA
N_CORES = 8
N_LEVELS = 4
F32 = mybir.dt.float32
BF16 = mybir.dt.bfloat16
NPBF16 = ml_dtypes.bfloat16

_cache: dict = {}


def _build(Bc: int, chunk: int, nchunks: int, warm: int = 9):
    """Build + compile the per-core Bass program for batch Bc = chunk*nchunks."""
    key = (Bc, chunk, nchunks, warm)
    if key in _cache:
        return _cache[key]

    nc = bacc.Bacc("TRN2", target_bir_lowering=False, debug=False,
                   num_devices=N_CORES)

    # Weights arrive host-linearized as [JG, 128, KT*512]:
    # element (jg, p, kt, jc) = W[kt*128 + p, jg*512 + jc], so each DMA
    # reads long contiguous runs per partition.  x / out are
    # partition-major [128, KT|JT, Bc] so any k-quarter slice is a
    # multi-KB contiguous run per partition (fat DMA packets hold their
    # own in the SDMA round-robin; feature-major layouts degrade to
    # ~0.5KB runs once column-sliced).
    xTv = nc.dram_tensor("xT", [128, KT, Bc], BF16, kind="ExternalInput")
    Ws = [nc.dram_tensor(f"W{l}", [JG, 128, KT * 512], BF16,
                         kind="ExternalInput")
          for l in range(N_LEVELS)]
    bias = nc.dram_tensor("bias", [N_LEVELS, D], F32, kind="ExternalInput")
    outv = nc.dram_tensor("out", [128, JT, Bc], BF16, kind="ExternalOutput")

    bv = bias.rearrange("l (jt p) -> p l jt", p=128)
    NQ = 4                      # W DMA split: 4 quarters of 4 k-tiles
    KQ = KT // NQ               # k-tiles per quarter
    QW = KQ * 512               # W free-dim elements per quarter
    PACE_WIN = 3                # max in-flight paced DMAs on the SP ring

    with tile.TileContext(nc) as tc:
        with (
            tc.tile_pool(name="acts", bufs=1) as acts,
            tc.tile_pool(name="w", bufs=4) as wpool,
            tc.tile_pool(name="ps", bufs=8, space="PSUM") as ps,
            tc.tile_pool(name="misc", bufs=1) as misc,
        ):
            actA = acts.tile([128, KT, Bc], BF16, tag="A")
            actB = acts.tile([128, KT, Bc], BF16, tag="B")
            btile = misc.tile([128, N_LEVELS, JT], F32)

            # Warm the PE HAM clock gate during the DMA lead-in: a short
            # burst of throwaway matmuls on a zeroed tile so the first
            # real matmul doesn't start on an idle->throttled PE.
            warmt = misc.tile([128, 512], BF16)
            nc.gpsimd.memset(warmt[:], 0.0)
            nc.gpsimd.dma_start(btile[:], bv)
            wacc = ps.tile([128, 512], F32, tag="ps", name="wacc")
            for _ in range(warm):
                nc.tensor.matmul(wacc[:], warmt[:, :128], warmt[:],
                                 start=True, stop=True)

            # Bulk weight DMAs go on the SP HWDGE ring, chained so at
            # most PACE_WIN are in flight.  The HW SDMA engines
            # round-robin packets across every queued transfer, so an
            # unbounded backlog makes every transfer finish near the
            # end; a short chain keeps completion order = consumption
            # order with the stream still running at full HBM rate.
            # x streams in parallel on the Activation HWDGE ring so the
            # two never serialize behind each other during the lead-in.
            paced = []

            def paced_dma(dst_ap, src_ap):
                h = nc.sync.dma_start(dst_ap, src_ap)
                if len(paced) >= PACE_WIN:
                    tile.add_dep_helper(h.ins, paced[-PACE_WIN].ins,
                                        reason="dma pacing chain")
                paced.append(h)
                return h

            # Lead-in: the first matmul chain needs x(k-quarter 0,
            # chunk 0) and W0[jg0][q0].  Ship the W piece on the Act
            # HWDGE ring (its 4KB-run packets hold their own in the
            # SDMA round-robin) in parallel with x on the SP ring, so
            # neither serializes behind the other.
            wt_first = wpool.tile([128, KT, 4 * 128], BF16, tag="w")
            wf_first = wt_first.rearrange("p kt j -> p (kt j)")
            nc.scalar.dma_start(wf_first[:, 0:QW], Ws[0][0][:, 0:QW])

            # x pieces (full-Bc k-slices, so every run stays contiguous),
            # emitted interleaved with the first weight quarters in
            # consumption-priority order.  First two pieces are k-tile
            # pairs so the first matmul chain can start early.
            pend_x = [(slice(2, 4),)] + [
                (slice(q * KQ, (q + 1) * KQ),) for q in range(1, NQ)]
            paced_dma(actA[:, 0:2, :], xTv[:, 0:2, :])

            for l in range(N_LEVELS):
                src = actA if l % 2 == 0 else actB
                dst = actB if l % 2 == 0 else actA
                for jg in range(JG):
                    first_jg = (l == 0 and jg == 0)
                    wt = wt_first if first_jg else wpool.tile(
                        [128, KT, 4 * 128], BF16, tag="w")
                    wflat = wt.rearrange("p kt j -> p (kt j)")
                    accs = [ps.tile([128, chunk], F32, tag="ps", name="acc")
                            for _ in range(nchunks * 4)]
                    last_jg = (l == N_LEVELS - 1 and jg == JG - 1)
                    for q in range(NQ):
                        if not (first_jg and q == 0):
                            paced_dma(
                                wflat[:, q * QW:(q + 1) * QW],
                                Ws[l][jg][:, q * QW:(q + 1) * QW])
                        if first_jg and pend_x:
                            (ks,) = pend_x.pop(0)
                            paced_dma(actA[:, ks, :], xTv[:, ks, :])
                        if last_jg:
                            continue  # emitted per-acc below
                        for c in range(nchunks):
                            cs = slice(c * chunk, (c + 1) * chunk)
                            for jj in range(4):
                                acc = accs[c * 4 + jj]
                                for kt in range(q * KQ, (q + 1) * KQ):
                                    nc.tensor.matmul(
                                        acc[:],
                                        wt[:, kt, jj * 128:(jj + 1) * 128],
                                        src[:, kt, cs],
                                        start=(kt == 0),
                                        stop=(kt == KT - 1),
                                    )
                    if not last_jg:
                        for c in range(nchunks):
                            cs = slice(c * chunk, (c + 1) * chunk)
                            for jj in range(4):
                                jt = jg * 4 + jj
                                acc = accs[c * 4 + jj]
                                nc.scalar.activation(
                                    dst[:, jt, cs], acc[:],
                                    mybir.ActivationFunctionType.Relu,
                                    bias=btile[:, l, jt:jt + 1],
                                )
                        if l == N_LEVELS - 1:
                            # final level, non-last jg: ship this jg's four
                            # feature blocks via SWDGE (GpSimd) so the
                            # store never head-of-line-blocks the paced W
                            # chain.
                            nc.gpsimd.dma_start(
                                outv[:, jg * 4:(jg + 1) * 4, :],
                                dst[:, jg * 4:(jg + 1) * 4, :])
                    else:
                        # Last jg of the last level: run each accumulation
                        # chain to completion so its activation + store
                        # overlap the remaining chains, shrinking the tail
                        # after the final matmul to ~1us.  The very last
                        # chain's activation + store are split in halves
                        # so the store of half 0 overlaps the activation
                        # of half 1; final stores ride the (now idle) SP
                        # HWDGE ring.
                        for c in range(nchunks):
                            cs = slice(c * chunk, (c + 1) * chunk)
                            for jj in range(4):
                                acc = accs[c * 4 + jj]
                                for kt in range(KT):
                                    nc.tensor.matmul(
                                        acc[:],
                                        wt[:, kt, jj * 128:(jj + 1) * 128],
                                        src[:, kt, cs],
                                        start=(kt == 0),
                                        stop=(kt == KT - 1),
                                    )
                                jt = jg * 4 + jj
                                last = (c == nchunks - 1 and jj == 3)
                                if not last:
                                    nc.scalar.activation(
                                        dst[:, jt, cs], acc[:],
                                        mybir.ActivationFunctionType.Relu,
                                        bias=btile[:, l, jt:jt + 1],
                                    )
                                    nc.gpsimd.dma_start(outv[:, jt, cs],
                                                        dst[:, jt, cs])
                                else:
                                    half = (chunk // 2 + 1) // 2 * 2
                                    for h in range(2):
                                        hs = slice(h * half,
                                                   chunk if h else half)
                                        gcs = slice(c * chunk + hs.start,
                                                    c * chunk + hs.stop)
                                        nc.scalar.activation(
                                            dst[:, jt, gcs], acc[:, hs],
                                            mybir.ActivationFunctionType.Relu,
                                            bias=btile[:, l, jt:jt + 1],
                                        )
                                        nc.sync.dma_start(outv[:, jt, gcs],
                                                          dst[:, jt, gcs])

    nc.compile()
    _cache[key] = nc
    return nc


def _linearize_w(W: np.ndarray) -> np.ndarray:
    """[D, D] f32 -> bf16 [JG, 128, KT*512], (jg,p,kt,jc) = W[kt*128+p, jg*512+jc]."""
    return np.ascontiguousarray(
        W.astype(NPBF16).reshape(KT, 128, JG, 512).transpose(2, 1, 0, 3).reshape(
            JG, 128, KT * 512))


def _plan(path_mask: np.ndarray):
    pm = np.asarray(path_mask)
    e3 = (pm[:, 0] * 4 + pm[:, 1] * 2 + pm[:, 2]).astype(np.int64)
    counts = np.bincount(e3, minlength=N_CORES)
    maxg = int(max(counts.max(), 1))
    nchunks = max(1, math.ceil(maxg / 512))
    chunk = max(256, math.ceil(maxg / nchunks))
    chunk = min(512, (chunk + 1) // 2 * 2)
    Bc = chunk * nchunks
    return e3, maxg, Bc, chunk, nchunks


def kernel(x, path_mask, W0, b0, W1, b1, W2, b2, W3, b3, _trace=False):
    x = np.ascontiguousarray(np.asarray(x, dtype=np.float32))
    Wls = [np.asarray(W, dtype=np.float32) for W in (W0, W1, W2, W3)]
    bls = [np.asarray(b, dtype=np.float32) for b in (b0, b1, b2, b3)]
    B = x.shape[0]

    e3, maxg, Bc, chunk, nchunks = _plan(path_mask)
    if Bc > 672:
        # extreme routing skew: SBUF can't hold the activations in one
        # pass; fall back to multiple 512-row passes per core.
        Bc, chunk, nchunks = 512, 512, 1
    nseg = math.ceil(maxg / Bc)
    nc = _build(Bc, chunk, nchunks)

    core_rows = [np.nonzero(e3 == c)[0] for c in range(N_CORES)]
    wb_maps = []
    for c in range(N_CORES):
        eids = (0, c >> 2, c >> 1, c)
        wb_maps.append({
            **{f"W{l}": _linearize_w(Wls[l][eids[l]])
               for l in range(N_LEVELS)},
            "bias": np.ascontiguousarray(
                np.stack([bls[l][eids[l]] for l in range(N_LEVELS)])),
        })

    out_full = np.zeros((B, D), dtype=np.float32)
    last_res = None
    for s in range(nseg):
        in_maps = []
        for c in range(N_CORES):
            rows = core_rows[c][s * Bc:(s + 1) * Bc]
            # partition-major [128, KT, Bc]: xTc[p, kt, b] = x[b, kt*128+p]
            xTc = np.zeros((128, KT, Bc), dtype=NPBF16)
            xTc[:, :, :len(rows)] = x[rows].astype(NPBF16).T.reshape(
                KT, 128, len(rows)).transpose(1, 0, 2)
            in_maps.append({"xT": xTc, **wb_maps[c]})
        res = run_bass_kernel_spmd(nc, in_maps, list(range(N_CORES)),
                                   trace=_trace)
        last_res = res
        for c in range(N_CORES):
            rows = core_rows[c][s * Bc:(s + 1) * Bc]
            oc = res.results[c]["out"]  # [128, JT, Bc], (p, jt, b)
            out_full[rows] = oc[:, :, :len(rows)].transpose(1, 0, 2).reshape(
                D, len(rows)).T.astype(np.float32)
    if _trace:
        return out_full, last_res
    return out_full


# revision 18
# speedup vs baseline: 1.0585x; 1.0074x over previous
"""Binary-tree gated-expert MoE kernel for 8 Trainium2 NeuronCores.

Reference computation (B=4096, D=2048, 4 levels, 1/2/4/8 experts):
    h = x
    for level l: h = relu(h @ Wl[eid_l] + bl[eid_l])
where eid_l is the l-bit prefix of the 3-bit leaf id built from
path_mask[:, 0:3].

Strategy: expert-parallel over the 8 leaves with host-side dispatch.
Sorting samples by leaf id makes every level's expert groups contiguous
(level-l ids are prefixes of the leaf id), so core c processes leaf
group c and needs exactly 4 weight matrices: W0[0], W1[c>>2], W2[c>>1],
W3[c].  Groups are Binomial(B, 1/8) ~ 512+-21 rows; each is padded to a
common per-core batch Bc.  On device each core runs 4 chained
matmul+relu levels in feature-major layout (activations stay transposed
[D, Bc] in SBUF across all levels; output partition dim = output
features, so no transposes anywhere).

Everything runs in bfloat16 (weights, activations, output) with fp32
PSUM accumulation: same 1 cycle/column PE rate as float32r, but half
the HBM traffic (33.5 MB/core of weights vs 67) and hardware fast
weight load, so LDWEIGHTS hides completely under the matmul stream.
End-to-end quantization error ~5e-3 vs the 2e-2 gate.  Weights stream
HBM->SBUF per 512-column group, paced + multi-buffered.
"""

import math

import numpy as np
import ml_dtypes

from concourse import bacc, mybir, tile
from concourse.bass_utils import run_bass_kernel_spmd

D = 2048
KT = D // 128          # 16 contraction k-tiles
JT = D // 128          # 16 output-feature blocks
JG = 4                 # j-groups of 4 blocks (512 features) per W DMA
N_CORES = 8
N_LEVELS = 4
F32 = mybir.dt.float32
BF16 = mybir.dt.bfloat16
NPBF16 = ml_dtypes.bfloat16

_cache: dict = {}


def _build(Bc: int, chunk: int, nchunks: int, warm: int = 6):
    """Build + compile the per-core Bass program for batch Bc = chunk*nchunks."""
    key = (Bc, chunk, nchunks, warm)
    if key in _cache:
        return _cache[key]

    nc = bacc.Bacc("TRN2", target_bir_lowering=False, debug=False,
                   num_devices=N_CORES)

    # Weights arrive host-linearized as [JG, 128, KT*512]:
    # element (jg, p, kt, jc) = W[kt*128 + p, jg*512 + jc], so each DMA
    # reads long contiguous runs per partition.  x / out are
    # partition-major [128, KT|JT, Bc] so any k-quarter slice is a
    # multi-KB contiguous run per partition (fat DMA packets hold their
    # own in the SDMA round-robin; feature-major layouts degrade to
    # ~0.5KB runs once column-sliced).
    xTv = nc.dram_tensor("xT", [128, KT, Bc], BF16, kind="ExternalInput")
    Ws = [nc.dram_tensor(f"W{l}", [JG, 128, KT * 512], BF16,
                         kind="ExternalInput")
          for l in range(N_LEVELS)]
    bias = nc.dram_tensor("bias", [N_LEVELS, D], F32, kind="ExternalInput")
    outv = nc.dram_tensor("out", [128, JT, Bc], BF16, kind="ExternalOutput")

    bv = bias.rearrange("l (jt p) -> p l jt", p=128)
    NQ = 4                      # W DMA split: 4 quarters of 4 k-tiles
    KQ = KT // NQ               # k-tiles per quarter
    QW = KQ * 512               # W free-dim elements per quarter
    PACE_WIN = 3                # max in-flight paced DMAs on the SP ring

    with tile.TileContext(nc) as tc:
        with (
            tc.tile_pool(name="acts", bufs=1) as acts,
            tc.tile_pool(name="w", bufs=4) as wpool,
            tc.tile_pool(name="ps", bufs=8, space="PSUM") as ps,
            tc.tile_pool(name="misc", bufs=1) as misc,
        ):
            actA = acts.tile([128, KT, Bc], BF16, tag="A")
            actB = acts.tile([128, KT, Bc], BF16, tag="B")
            btile = misc.tile([128, N_LEVELS, JT], F32)

            # Warm the PE HAM clock gate during the DMA lead-in: a short
            # burst of throwaway matmuls on a zeroed tile so the first
            # real matmul doesn't start on an idle->throttled PE.
            warmt = misc.tile([128, 512], BF16)
            nc.gpsimd.memset(warmt[:], 0.0)
            wacc = ps.tile([128, 512], F32, tag="ps", name="wacc")
            for _ in range(warm):
                nc.tensor.matmul(wacc[:], warmt[:, :128], warmt[:],
                                 start=True, stop=True)

            # Bulk weight DMAs go on the SP HWDGE ring, chained so at
            # most PACE_WIN are in flight.  The HW SDMA engines
            # round-robin packets across every queued transfer, so an
            # unbounded backlog makes every transfer finish near the
            # end; a short chain keeps completion order = consumption
            # order with the stream still running at full HBM rate.
            # x streams in parallel on the Activation HWDGE ring so the
            # two never serialize behind each other during the lead-in.
            paced = []

            def paced_dma(dst_ap, src_ap):
                h = nc.sync.dma_start(dst_ap, src_ap)
                if len(paced) >= PACE_WIN:
                    tile.add_dep_helper(h.ins, paced[-PACE_WIN].ins,
                                        reason="dma pacing chain")
                paced.append(h)
                return h

            # Lead-in, in exact consumption order of the kt-major
            # first-jg matmul stream: alternate x k-tile-pairs and W
            # k-tile-pair pieces on the SP ring, so the first matmul can
            # start after ~540KB instead of ~1.1MB.  Bulk x mid-quarters
            # ride the otherwise-idle Act HWDGE ring, due just-in-time
            # (its effective rate is ~2x lower when SP is busy).
            wt_first = wpool.tile([128, KT, 4 * 128], BF16, tag="w")
            wf_first = wt_first.rearrange("p kt j -> p (kt j)")
            paced_dma(actA[:, 0:2, :], xTv[:, 0:2, :])
            paced_dma(wf_first[:, 0:2 * 512], Ws[0][0][:, 0:2 * 512])
            paced_dma(actA[:, 2:4, :], xTv[:, 2:4, :])
            paced_dma(wf_first[:, 2 * 512:4 * 512],
                      Ws[0][0][:, 2 * 512:4 * 512])
            nc.scalar.dma_start(actA[:, 4:8, :], xTv[:, 4:8, :])
            nc.scalar.dma_start(actA[:, 8:12, :], xTv[:, 8:12, :])
            nc.scalar.dma_start(btile[:], bv)

            for l in range(N_LEVELS):
                src = actA if l % 2 == 0 else actB
                dst = actB if l % 2 == 0 else actA
                for jg in range(JG):
                    first_jg = (l == 0 and jg == 0)
                    wt = wt_first if first_jg else wpool.tile(
                        [128, KT, 4 * 128], BF16, tag="w")
                    wflat = wt.rearrange("p kt j -> p (kt j)")
                    accs = [ps.tile([128, chunk], F32, tag="ps", name="acc")
                            for _ in range(nchunks * 4)]
                    last_jg = (l == N_LEVELS - 1 and jg == JG - 1)
                    for q in range(NQ):
                        if not (first_jg and q == 0):
                            paced_dma(
                                wflat[:, q * QW:(q + 1) * QW],
                                Ws[l][jg][:, q * QW:(q + 1) * QW])
                        if first_jg and q == NQ - 1:
                            # last x quarter closes the SP lead-in
                            paced_dma(actA[:, 12:16, :], xTv[:, 12:16, :])
                        if last_jg or first_jg:
                            continue  # emitted kt-major / per-acc below
                        for c in range(nchunks):
                            cs = slice(c * chunk, (c + 1) * chunk)
                            for jj in range(4):
                                acc = accs[c * 4 + jj]
                                for kt in range(q * KQ, (q + 1) * KQ):
                                    nc.tensor.matmul(
                                        acc[:],
                                        wt[:, kt, jj * 128:(jj + 1) * 128],
                                        src[:, kt, cs],
                                        start=(kt == 0),
                                        stop=(kt == KT - 1),
                                    )
                    if first_jg:
                        # kt-major so the PE consumes the lead-in pieces
                        # in exactly their arrival order.
                        for kt in range(KT):
                            for c in range(nchunks):
                                cs = slice(c * chunk, (c + 1) * chunk)
                                for jj in range(4):
                                    acc = accs[c * 4 + jj]
                                    nc.tensor.matmul(
                                        acc[:],
                                        wt[:, kt, jj * 128:(jj + 1) * 128],
                                        src[:, kt, cs],
                                        start=(kt == 0),
                                        stop=(kt == KT - 1),
                                    )
                    if not last_jg:
                        for c in range(nchunks):
                            cs = slice(c * chunk, (c + 1) * chunk)
                            for jj in range(4):
                                jt = jg * 4 + jj
                                acc = accs[c * 4 + jj]
                                nc.scalar.activation(
                                    dst[:, jt, cs], acc[:],
                                    mybir.ActivationFunctionType.Relu,
                                    bias=btile[:, l, jt:jt + 1],
                                )
                        if l == N_LEVELS - 1:
                            # final level, non-last jg: ship this jg's four
                            # feature blocks via SWDGE (GpSimd) so the
                            # store never head-of-line-blocks the paced W
                            # chain.
                            nc.gpsimd.dma_start(
                                outv[:, jg * 4:(jg + 1) * 4, :],
                                dst[:, jg * 4:(jg + 1) * 4, :])
                    else:
                        # Last jg of the last level: run each accumulation
                        # chain to completion so its activation + store
                        # overlap the remaining chains, shrinking the tail
                        # after the final matmul to ~1us.  The very last
                        # chain's activation + store are split in halves
                        # so the store of half 0 overlaps the activation
                        # of half 1; final stores ride the (now idle) SP
                        # HWDGE ring.
                        for c in range(nchunks):
                            cs = slice(c * chunk, (c + 1) * chunk)
                            for jj in range(4):
                                acc = accs[c * 4 + jj]
                                for kt in range(KT):
                                    nc.tensor.matmul(
                                        acc[:],
                                        wt[:, kt, jj * 128:(jj + 1) * 128],
                                        src[:, kt, cs],
                                        start=(kt == 0),
                                        stop=(kt == KT - 1),
                                    )
                                jt = jg * 4 + jj
                                last = (c == nchunks - 1 and jj == 3)
                                if not last:
                                    nc.scalar.activation(
                                        dst[:, jt, cs], acc[:],
                                        mybir.ActivationFunctionType.Relu,
                                        bias=btile[:, l, jt:jt + 1],
                                    )
                                    nc.gpsimd.dma_start(outv[:, jt, cs],
                                                        dst[:, jt, cs])
                                else:
                                    half = (chunk // 2 + 1) // 2 * 2
                                    for h in range(2):
                                        hs = slice(h * half,
                                                   chunk if h else half)
                                        gcs = slice(c * chunk + hs.start,
                                                    c * chunk + hs.stop)
                                        nc.scalar.activation(
                                            dst[:, jt, gcs], acc[:, hs],
                                            mybir.ActivationFunctionType.Relu,
                                            bias=btile[:, l, jt:jt + 1],
                                        )
                                        nc.sync.dma_start(outv[:, jt, gcs],
                                                          dst[:, jt, gcs])

    nc.compile()
    _cache[key] = nc
    return nc


def _linearize_w(W: np.ndarray) -> np.ndarray:
    """[D, D] f32 -> bf16 [JG, 128, KT*512], (jg,p,kt,jc) = W[kt*128+p, jg*512+jc]."""
    return np.ascontiguousarray(
        W.astype(NPBF16).reshape(KT, 128, JG, 512).transpose(2, 1, 0, 3).reshape(
            JG, 128, KT * 512))


def _plan(path_mask: np.ndarray):
    pm = np.asarray(path_mask)
    e3 = (pm[:, 0] * 4 + pm[:, 1] * 2 + pm[:, 2]).astype(np.int64)
    counts = np.bincount(e3, minlength=N_CORES)
    maxg = int(max(counts.max(), 1))
    nchunks = max(1, math.ceil(maxg / 512))
    chunk = max(256, math.ceil(maxg / nchunks))
    chunk = min(512, (chunk + 1) // 2 * 2)
    Bc = chunk * nchunks
    return e3, maxg, Bc, chunk, nchunks


def kernel(x, path_mask, W0, b0, W1, b1, W2, b2, W3, b3, _trace=False):
    x = np.ascontiguousarray(np.asarray(x, dtype=np.float32))
    Wls = [np.asarray(W, dtype=np.float32) for W in (W0, W1, W2, W3)]
    bls = [np.asarray(b, dtype=np.float32) for b in (b0, b1, b2, b3)]
    B = x.shape[0]

    e3, maxg, Bc, chunk, nchunks = _plan(path_mask)
    if Bc > 672:
        # extreme routing skew: SBUF can't hold the activations in one
        # pass; fall back to multiple 512-row passes per core.
        Bc, chunk, nchunks = 512, 512, 1
    nseg = math.ceil(maxg / Bc)
    nc = _build(Bc, chunk, nchunks)

    core_rows = [np.nonzero(e3 == c)[0] for c in range(N_CORES)]
    wb_maps = []
    for c in range(N_CORES):
        eids = (0, c >> 2, c >> 1, c)
        wb_maps.append({
            **{f"W{l}": _linearize_w(Wls[l][eids[l]])
               for l in range(N_LEVELS)},
            "bias": np.ascontiguousarray(
                np.stack([bls[l][eids[l]] for l in range(N_LEVELS)])),
        })

    out_full = np.zeros((B, D), dtype=np.float32)
    last_res = None
    for s in range(nseg):
        in_maps = []
        for c in range(N_CORES):
            rows = core_rows[c][s * Bc:(s + 1) * Bc]
            # partition-major [128, KT, Bc]: xTc[p, kt, b] = x[b, kt*128+p]
            xTc = np.zeros((128, KT, Bc), dtype=NPBF16)
            xTc[:, :, :len(rows)] = x[rows].astype(NPBF16).T.reshape(
                KT, 128, len(rows)).transpose(1, 0, 2)
            in_maps.append({"xT": xTc, **wb_maps[c]})
        res = run_bass_kernel_spmd(nc, in_maps, list(range(N_CORES)),
                                   trace=_trace)
        last_res = res
        for c in range(N_CORES):
            rows = core_rows[c][s * Bc:(s + 1) * Bc]
            oc = res.results[c]["out"]  # [128, JT, Bc], (p, jt, b)
            out_full[rows] = oc[:, :, :len(rows)].transpose(1, 0, 2).reshape(
                D, len(rows)).T.astype(np.float32)
    if _trace:
        return out_full, last_res
    return out_full
